# revision 29
# baseline (speedup 1.0000x reference)
"""Trainium2 Bass kernel for gated GNN message passing (8 NeuronCores, SPMD).

Reference computation:
    k = x @ W_key + b_key;  q = x @ W_query + b_query;  v = x @ W_value + b_value
    e = edge_attr @ W_edge + b_edge
    gate = sigmoid(k[dst] + q[src] + e)
    msg = gate * v[src]
    agg = segment_sum(msg, dst, N)
    out = relu(agg + x @ W_skip + b_skip)
    return (out, edge_attr, u)

Distribution: nodes (and their in-edges, partitioned by destination) are
sharded across 8 cores; weights replicated; source-node features fetched via
an on-device dma_gather from a per-core replica of a (q|v) table built on
device.  No collectives are needed.

Device pipeline per core (edge-major, bf16 compute / f32 accumulate):
  phase 0: qv table [N,128] f32 in DRAM  (q|v rows, built from x via PE)
           k-blocks (+combined gate bias) and skip-blocks (+b_skip) in SBUF
  phase 1: edges grouped by (dst-block of 128 nodes, src window of 25056) and
           padded to 128-edge tiles; per (super-block, window) run one
           dma_gather of qv[src] rows (int16 idx, 4 SWDGE queues);
           per 128-edge tile: e+kd accumulated in PSUM via two matmuls
           (edge_attr^T half-split stationary; node-indicator x k_blk);
           gate = sigmoid(psum + q); msg = gate * v;
           agg += ind_T^T @ msg  (indicator matmul segment sum);
           per super-block: out rows = relu(agg + skip) -> DRAM.
"""

import sys

import numpy as np

try:
    import concourse.bass  # noqa: F401
except ImportError:
    sys.path.insert(0, "/opt/trn_rl_repo")

# ---------------------------------------------------------------- constants
N, E, IN_C, OUT_C, EDGE_D = 100000, 1600000, 128, 64, 64
NCORES = 8
BLK = 128            # dst nodes per block
SUPER = 4            # dst blocks per super-block (gather batching)
NWIN = 4             # source-index windows (int16 reach 32768)
WIN_STRIDE = 25056   # window w covers src in [w*WIN_STRIDE, w*WIN_STRIDE+32768)
WIN_SIZE = 32768
CHUNK_TILES = 8      # tiles per elementwise chunk
NSWQ = 4             # SWDGE queues for dma_gather

_graph_cache = {}
SIM_MODE = False
SIM_NCORES = 1
TRACE = False
LAST_RESULT = None


def _ceil_div(a, b):
    return -(-a // b)


# =================================================================== schedule
def _make_schedule(src, dst, n_nodes, n_cores):
    """Partition edges by destination across cores; build the shared static
    schedule (same instruction stream for every core) plus per-core data."""
    npc = n_nodes // n_cores            # nodes per core
    nblk = _ceil_div(npc, BLK)          # dst blocks per core
    nsuper = _ceil_div(nblk, SUPER)

    order = np.argsort(dst, kind="stable")
    dst_s = dst[order]
    src_s = src[order]

    core_lo = np.searchsorted(dst_s, np.arange(n_cores) * npc)
    core_hi = np.searchsorted(dst_s, (np.arange(n_cores) + 1) * npc)

    # per-core per-block edge lists sorted by src, with window boundary CDFs.
    # Window w accepts src in [w*WIN_STRIDE, w*WIN_STRIDE + WIN_SIZE); edges in
    # the overlap zones are assigned flexibly so that windows 0..NWIN-2 hold
    # exact multiples of 128 edges (minimal padding).
    blk_edges = [[None] * nblk for _ in range(n_cores)]   # src-sorted edge ids
    cnt_cb = np.zeros((n_cores, nblk), dtype=np.int64)
    Lc = np.zeros((n_cores, nblk, NWIN + 1), dtype=np.int64)  # srcs < w*STRIDE
    for c in range(n_cores):
        lo, hi = core_lo[c], core_hi[c]
        d_loc = dst_s[lo:hi] - c * npc
        s_loc = src_s[lo:hi]
        blk_id = d_loc // BLK
        ordk = np.argsort(blk_id * (n_nodes + 1) + s_loc, kind="stable")
        keys_sorted = blk_id[ordk]
        bounds = np.searchsorted(keys_sorted, np.arange(nblk + 1))
        for b in range(nblk):
            sel = lo + ordk[bounds[b]:bounds[b + 1]]
            blk_edges[c][b] = sel
            cnt_cb[c, b] = len(sel)
            srcs = src_s[sel]
            for w in range(NWIN + 1):
                Lc[c, b, w] = np.searchsorted(srcs, w * WIN_STRIDE)

    T_b = _ceil_div(cnt_cb.max(axis=0), BLK)              # tiles per block
    # uniform cumulative tile cut points C[b, w], w = 0..NWIN-1
    t_bw = np.zeros((nblk, NWIN), dtype=np.int64)
    for b in range(nblk):
        C_prev = 0
        for w in range(NWIN - 1):
            C_w = max(int(_ceil_div(Lc[:, b, w + 1].max(), BLK)), C_prev)
            C_w = min(C_w, int(T_b[b]))
            t_bw[b, w] = C_w - C_prev
            C_prev = C_w
        t_bw[b, NWIN - 1] = int(T_b[b]) - C_prev

    # per-core slot filling: window w takes up to 128*t_bw[b,w] eligible edges
    # (src < w*WIN_STRIDE + WIN_SIZE) in src order; shortfall = dummy slots.
    groups = [[[None] * NWIN for _ in range(nblk)] for _ in range(n_cores)]
    for c in range(n_cores):
        for b in range(nblk):
            sel = blk_edges[c][b]
            srcs = src_s[sel]
            ptr = 0
            for w in range(NWIN):
                slots = BLK * int(t_bw[b, w])
                hi_b = w * WIN_STRIDE + WIN_SIZE
                n_elig = int(np.searchsorted(srcs, hi_b)) - ptr
                take = min(slots, n_elig)
                if ptr < len(srcs) and take < min(slots, len(srcs) - ptr):
                    # leftover edges below next window base would be dropped
                    assert srcs[ptr + take] >= (w + 1) * WIN_STRIDE, (
                        "window packing infeasible")
                groups[c][b][w] = sel[ptr:ptr + take]
                ptr += take
            assert ptr == len(sel), (c, b, ptr, len(sel))

    runs = []          # (super, window, n_tiles)
    tile_block = []    # local dst-block per tile
    nt = 0
    for s in range(nsuper):
        blocks = range(s * SUPER, min((s + 1) * SUPER, nblk))
        for w in range(NWIN):
            rt = 0
            for b in blocks:
                tb = int(t_bw[b, w])
                tile_block.extend([b] * tb)
                nt += tb
                rt += tb
            runs.append((s, w, rt))
    tile_block = np.asarray(tile_block, dtype=np.int64)

    # chunks: groups of <=CHUNK_TILES tiles within each run
    chunks = []        # (run_idx, gt0, k)
    g0 = 0
    for ri, (s, w, rt) in enumerate(runs):
        for j0 in range(0, rt, CHUNK_TILES):
            chunks.append((ri, g0 + j0, min(CHUNK_TILES, rt - j0)))
        g0 += rt

    return dict(
        npc=npc, nblk=nblk, nsuper=nsuper, n_nodes=n_nodes,
        runs=runs, tile_block=tile_block, NT=nt, t_bw=t_bw, chunks=chunks,
        groups=groups, order=order, src_s=src_s, dst_s=dst_s,
    )


def _pack_core(sched, c, edge_attr_s):
    """Build one core's input arrays (contiguous pre-tiled layouts)."""
    npc, nblk = sched["npc"], sched["nblk"]
    NT = sched["NT"]
    EPAD = NT * BLK
    t_bw = sched["t_bw"]
    groups = sched["groups"]
    src_s, dst_s = sched["src_s"], sched["dst_s"]
    runs, chunks = sched["runs"], sched["chunks"]

    slot_src = np.zeros(EPAD, dtype=np.int64)
    slot_off = np.full(EPAD, 200.0, dtype=np.float32)
    slot_edge = np.full(EPAD, -1, dtype=np.int64)

    pos = 0
    for s in range(sched["nsuper"]):
        blocks = range(s * SUPER, min((s + 1) * SUPER, nblk))
        for w in range(NWIN):
            for b in blocks:
                sel = groups[c][b][w]
                n = len(sel)
                tb = int(t_bw[b, w])
                slot_src[pos:pos + n] = src_s[sel]
                slot_off[pos:pos + n] = (dst_s[sel] - c * npc - b * BLK).astype(np.float32)
                slot_edge[pos:pos + n] = sel
                slot_src[pos + n:pos + tb * BLK] = min(w * WIN_STRIDE,
                                                       sched["n_nodes"] - 1)
                pos += tb * BLK
    assert pos == EPAD

    # ---- gather indices: per-run contiguous [128, n/16] C-order blocks ----
    idx_words = sum(128 * (rt * BLK // 16) for (_, _, rt) in runs)
    idx16 = np.zeros(idx_words, dtype=np.int16)
    off = 0
    pos = 0
    for (s, w, rt) in runs:
        n = rt * BLK
        if n:
            local = (slot_src[pos:pos + n] - w * WIN_STRIDE).astype(np.int64)
            assert local.min() >= 0 and local.max() < WIN_SIZE
            wrapped = local.reshape(n // 16, 16).T.astype(np.int16)   # [16, n/16]
            block = np.tile(wrapped, (8, 1))                          # [128, n/16]
            idx16[off:off + 128 * (n // 16)] = block.ravel()
            off += 128 * (n // 16)
        pos += n
    assert off == idx_words and pos == EPAD

    # ---- dst offsets: per-run contiguous [128, rt] C-order ---------------
    dofr = np.zeros(128 * NT, dtype=np.float32)
    offv = slot_off.reshape(NT, BLK).T        # [128, NT]
    off = 0
    g0 = 0
    for (s, w, rt) in runs:
        if rt:
            dofr[off:off + 128 * rt] = offv[:, g0:g0 + rt].ravel()
            off += 128 * rt
        g0 += rt
    dstrow = slot_off.reshape(1, EPAD)        # edge-major (broadcast source)

    # ---- edge_attr: per-chunk [128, 256] half-split contiguous ----------
    nchunk = len(chunks)
    eac = np.zeros((nchunk * 128, (CHUNK_TILES // 2) * BLK), dtype=np.float32)
    valid = slot_edge >= 0
    ea_rows = np.zeros((EPAD, EDGE_D), dtype=np.float32)
    ea_rows[valid] = edge_attr_s[slot_edge[valid]]
    eaT = ea_rows.reshape(NT, BLK, EDGE_D).transpose(0, 2, 1)  # [NT, 64ch, 128p]
    for ci, (ri, gt0, k) in enumerate(chunks):
        blkv = eac[128 * ci:128 * ci + 128].reshape(128, CHUNK_TILES // 2, BLK)
        for j in range(k):
            blkv[64 * (j % 2):64 * (j % 2) + 64, j // 2, :] = eaT[gt0 + j]

    return dict(
        idx16=idx16.reshape(1, -1),
        dofr=dofr.reshape(1, -1),
        dstrow=dstrow,
        eac=eac,
        EPAD=EPAD,
    )


# ================================================================ bass graph
def _build_graph(cfg):
    import concourse.bass as bass  # noqa: F401
    from concourse import bacc
    import concourse.mybir as mybir
    from concourse.tile import TileContext

    f32 = mybir.dt.float32
    bf16 = mybir.dt.bfloat16
    i32 = mybir.dt.int32
    i16 = mybir.dt.int16
    AOT = mybir.AluOpType
    AFT = mybir.ActivationFunctionType

    NT = cfg["NT"]
    nblk = cfg["nblk"]
    runs = cfg["runs"]
    chunks = cfg["chunks"]
    tile_block = cfg["tile_block"]
    NPADT = cfg["NPADT"]
    NPADL = nblk * BLK
    idx_words = cfg["idx_words"]
    nchunk = len(chunks)
    zero_bias = cfg["zero_bias"]

    nc = bacc.Bacc(num_swdge_queues=NSWQ)

    # ---------------- dram parameters ----------------
    xt_t = nc.declare_dram_parameter("xt_full", [128, NPADT], bf16, isOutput=False)
    xl_t = nc.declare_dram_parameter("xl_full", [128, NPADL], bf16, isOutput=False)
    wqv = nc.declare_dram_parameter("wqv", [128, 128], bf16, isOutput=False)
    wk = nc.declare_dram_parameter("wk", [128, 64], bf16, isOutput=False)
    wskip = nc.declare_dram_parameter("wskip", [128, 64], bf16, isOutput=False)
    wedge = nc.declare_dram_parameter("wedge", [64, 64], bf16, isOutput=False)
    gbias = nc.declare_dram_parameter("gbias", [1, 64], bf16, isOutput=False)
    sbias = nc.declare_dram_parameter("sbias", [1, 64], bf16, isOutput=False)
    qvbias = nc.declare_dram_parameter("qvbias", [1, 128], f32, isOutput=False)
    eac_d = nc.declare_dram_parameter("eac", [nchunk * 128, (CHUNK_TILES // 2) * BLK], bf16, isOutput=False)
    idx_d = nc.declare_dram_parameter("idx16", [1, idx_words], i16, isOutput=False)
    dofr_d = nc.declare_dram_parameter("dofr", [1, 128 * NT], f32, isOutput=False)
    drow_d = nc.declare_dram_parameter("dstrow", [1, NT * BLK], bf16, isOutput=False)
    out_d = nc.declare_dram_parameter("out", [NPADL, 64], f32, isOutput=True)

    qvtab = nc.dram_tensor("qvtab", [NPADT, 128], f32)

    # alternate HWDGE issue between SP and ACT sequencers
    dma_ctr = [0]

    def dma(out, in_):
        dma_ctr[0] += 1
        eng = nc.sync if dma_ctr[0] % 2 else nc.scalar
        return eng.dma_start(out=out, in_=in_)

    with TileContext(nc) as tc, tc.tile_pool(name="consts", bufs=1) as cpool:
        w_qv = cpool.tile([128, 128], bf16); nc.sync.dma_start(out=w_qv[:], in_=wqv[:])
        w_k = cpool.tile([128, 64], bf16); nc.sync.dma_start(out=w_k[:], in_=wk[:])
        w_s = cpool.tile([128, 64], bf16); nc.sync.dma_start(out=w_s[:], in_=wskip[:])
        w_e = cpool.tile([128, 64], bf16)
        nc.sync.dma_start(out=w_e[0:64, :], in_=wedge[:])
        nc.sync.dma_start(out=w_e[64:128, :], in_=wedge[:])
        gb = cpool.tile([1, 64], bf16); nc.sync.dma_start(out=gb[:], in_=gbias[:])
        sb_b = cpool.tile([1, 64], bf16); nc.sync.dma_start(out=sb_b[:], in_=sbias[:])
        ones1 = cpool.tile([1, 128], bf16); nc.vector.memset(ones1[:], 1.0)
        qvb = cpool.tile([128, 128], f32)
        nc.sync.dma_start(out=qvb[:], in_=qvbias[:].to_broadcast([128, 128]))
        iota_i = cpool.tile([128, 128], i32)
        nc.gpsimd.iota(iota_i[:], pattern=[[1, 128]], base=0, channel_multiplier=0)
        iota_row = cpool.tile([128, 128], bf16)
        nc.vector.tensor_copy(out=iota_row[:], in_=iota_i[:])
        ioc_i = cpool.tile([128, 1], i32)
        nc.gpsimd.iota(ioc_i[:], pattern=[[0, 1]], base=0, channel_multiplier=1)
        iota_col = cpool.tile([128, 1], bf16)
        nc.vector.tensor_copy(out=iota_col[:], in_=ioc_i[:])
        iota_colw = cpool.tile([128, CHUNK_TILES * 128], bf16)
        nc.vector.tensor_copy(
            out=iota_colw[:],
            in_=iota_col[:].to_broadcast([128, CHUNK_TILES * 128]))

        k_all = cpool.tile([128, 64 * nblk], bf16)
        skip_all = cpool.tile([128, 64 * nblk], f32)

        # ---------------- phase 0a: qv table (groups of 4 blocks) ----------
        tab_writes = []   # (row_hi, dma_inst)
        ntb = NPADT // 128
        with tc.tile_pool(name="p0", bufs=3) as p0, \
             tc.tile_pool(name="p0ps", bufs=3, space="PSUM") as p0ps:
            for tq in range(_ceil_div(ntb, 4)):
                t0 = tq * 4
                kt = min(4, ntb - t0)
                xb = p0.tile([128, 128 * kt], bf16, tag="xb")
                dma(xb[:], xt_t[:, 128 * t0:128 * (t0 + kt)])
                ps = p0ps.tile([128, 128 * kt], f32, tag="ps")
                for j in range(kt):
                    nc.tensor.matmul(ps[:, 128 * j:128 * (j + 1)],
                                     lhsT=xb[:, 128 * j:128 * (j + 1)],
                                     rhs=w_qv[:], start=True, stop=True)
                sbuf = p0.tile([128, 128 * kt], f32, tag="qvsb")
                if zero_bias:
                    if tq % 2 == 0:
                        nc.scalar.activation(out=sbuf[:], in_=ps[:], func=AFT.Copy)
                    else:
                        nc.vector.tensor_copy(out=sbuf[:], in_=ps[:])
                else:
                    for j in range(kt):
                        nc.vector.tensor_tensor(out=sbuf[:, 128 * j:128 * (j + 1)],
                                                in0=ps[:, 128 * j:128 * (j + 1)],
                                                in1=qvb[:], op=AOT.add)
                wi = dma(qvtab[128 * t0:128 * (t0 + kt), :].rearrange(
                    "(t n) c -> n t c", t=kt),
                    sbuf[:].rearrange("n (t c) -> n t c", t=kt))
                tab_writes.append((128 * (t0 + kt), wi))

        # per-window funnel markers: gathers of window w wait on all table
        # writes covering [w*WIN_STRIDE, w*WIN_STRIDE+WIN_SIZE)
        from concourse.tile import add_dep_helper
        win_markers = []
        mk_tile = cpool.tile([1, 8], f32)
        for w in range(NWIN):
            hi_w = min(w * WIN_STRIDE + WIN_SIZE, NPADT)
            mk = nc.vector.memset(mk_tile[0:1, w:w + 1], 0.0)
            for row_hi, wi in tab_writes:
                if True if row_hi <= hi_w else False:
                    add_dep_helper(mk.ins, wi.ins, sync=True,
                                   reason=f"qvtab window {w} ready")
            win_markers.append(mk)

        # ---------------- phase 0b: k & skip blocks (groups of 4) ----------
        with tc.tile_pool(name="p0b", bufs=3) as p0b, \
             tc.tile_pool(name="p0bps", bufs=2, space="PSUM") as p0bps:
            for bq in range(_ceil_div(nblk, 4)):
                b0 = bq * 4
                kb = min(4, nblk - b0)
                xb = p0b.tile([128, 128 * kb], bf16, tag="xlb")
                dma(xb[:], xl_t[:, 128 * b0:128 * (b0 + kb)])
                kps = p0bps.tile([128, 64 * kb], f32, tag="kps")
                sps = p0bps.tile([128, 64 * kb], f32, tag="sps")
                for j in range(kb):
                    nc.tensor.matmul(kps[:, 64 * j:64 * (j + 1)],
                                     lhsT=xb[:, 128 * j:128 * (j + 1)],
                                     rhs=w_k[:], start=True, stop=False)
                    nc.tensor.matmul(kps[:, 64 * j:64 * (j + 1)],
                                     lhsT=ones1[:], rhs=gb[:], start=False, stop=True)
                    nc.tensor.matmul(sps[:, 64 * j:64 * (j + 1)],
                                     lhsT=xb[:, 128 * j:128 * (j + 1)],
                                     rhs=w_s[:], start=True, stop=False)
                    nc.tensor.matmul(sps[:, 64 * j:64 * (j + 1)],
                                     lhsT=ones1[:], rhs=sb_b[:], start=False, stop=True)
                nc.vector.tensor_copy(out=k_all[:, 64 * b0:64 * (b0 + kb)], in_=kps[:])
                nc.scalar.activation(out=skip_all[:, 64 * b0:64 * (b0 + kb)],
                                     in_=sps[:], func=AFT.Copy)

        # ---------------- phase 1: edge pipeline ----------------
        first_tile_of_block = {}
        last_tile_of_block = {}
        for g, b in enumerate(tile_block):
            first_tile_of_block.setdefault(int(b), g)
            last_tile_of_block[int(b)] = g

        run_chunks = {}
        for ci, (ri, gt0, k) in enumerate(chunks):
            run_chunks.setdefault(ri, []).append((ci, gt0, k))

        with tc.tile_pool(name="gq", bufs=3) as gq, \
             tc.tile_pool(name="work", bufs=4) as work, \
             tc.tile_pool(name="ind", bufs=4) as indp, \
             tc.tile_pool(name="finp", bufs=2) as finp, \
             tc.tile_pool(name="eps", bufs=3, space="PSUM") as eps, \
             tc.tile_pool(name="aggps", bufs=1, space="PSUM") as aggps:

            agg_tiles = {}
            fin_tiles = {}
            idx_off = 0
            dof_off = 0
            g0 = 0
            qnum = 0
            for ri, (s, w, rt) in enumerate(runs):
                if rt == 0:
                    continue
                n_idx = rt * BLK
                ixt = work.tile([128, n_idx // 16], i16, tag="ixt")
                dma(ixt[:], idx_d[0, idx_off:idx_off + 128 * (n_idx // 16)].rearrange(
                    "(p c) -> p c", p=128))
                idx_off += 128 * (n_idx // 16)
                dofr_sb = work.tile([128, rt], f32, tag="dofr")
                dma(dofr_sb[:], dofr_d[0, dof_off:dof_off + 128 * rt].rearrange(
                    "(p c) -> p c", p=128))
                dof_off += 128 * rt

                qv_g = gq.tile([128, rt * 128], f32, tag="qvg")
                win_hi = min(w * WIN_STRIDE + WIN_SIZE, NPADT)
                # split the run's gather across the SWDGE queues so their
                # descriptor generation overlaps on the Q7 cores
                nsub = min(NSWQ, rt)
                sub = _ceil_div(rt, nsub)
                t0s = list(range(0, rt, sub))
                for si, st in enumerate(t0s):
                    en = min(st + sub, rt)
                    ni = (en - st) * BLK
                    gi = nc.gpsimd.dma_gather(
                        out_ap=qv_g[:, 128 * st:128 * en].rearrange(
                            "p (t d) -> p t d", t=en - st),
                        in_ap=qvtab[w * WIN_STRIDE:win_hi, :],
                        idxs_ap=ixt[:, 8 * st:8 * en],
                        num_idxs=ni, num_idxs_reg=ni, elem_size=128,
                        single_packet=False, queue_num=(qnum + si) % NSWQ)
                    from concourse.tile import add_dep_helper as _adh
                    _adh(gi.ins if hasattr(gi, 'ins') else gi, win_markers[w].ins, sync=True,
                         reason="gather after qvtab window ready")
                qnum += len(t0s)

                dbc = gq.tile([128, rt * 128], bf16, tag="dbc")
                dma(dbc[:], drow_d[:, BLK * g0:BLK * (g0 + rt)].to_broadcast(
                    [128, rt * 128]))

                ea_pair = {}
                rcs = run_chunks[ri]
                for pi in range(0, len(rcs), 2):
                    pcs = rcs[pi:pi + 2]
                    u = len(pcs)
                    et = work.tile([128, u * (CHUNK_TILES // 2) * BLK], bf16,
                                   tag="easb")
                    ci0 = pcs[0][0]
                    dma(et[:].rearrange("p (u c) -> p u c", u=u),
                        eac_d[128 * ci0:128 * (ci0 + u), :].rearrange(
                            "(u p) c -> p u c", u=u))
                    for ui, (ci, _, _) in enumerate(pcs):
                        ea_pair[ci] = (et, ui)
                for (ci, gt0, k) in run_chunks[ri]:
                    j0 = gt0 - g0
                    ind_n = indp.tile([128, 128 * k], bf16, tag="indn")
                    nc.vector.tensor_tensor(
                        out=ind_n[:], in0=dbc[:, 128 * j0:128 * (j0 + k)],
                        in1=iota_colw[:, :128 * k],
                        op=AOT.is_equal)
                    ind_T = indp.tile([128, 128 * k], bf16, tag="indt")
                    for j in range(k):
                        nc.vector.tensor_tensor(
                            out=ind_T[:, 128 * j:128 * (j + 1)], in0=iota_row[:],
                            in1=dofr_sb[:, j0 + j:j0 + j + 1].to_broadcast([128, 128]),
                            op=AOT.is_equal)

                    et, ui = ea_pair[ci]
                    ea_sb = et[:, ui * (CHUNK_TILES // 2) * BLK:
                               (ui + 1) * (CHUNK_TILES // 2) * BLK]

                    ek = eps.tile([128, 64 * k], f32, tag="ek")
                    for j in range(k):
                        g = gt0 + j
                        b = int(tile_block[g])
                        pb = 64 * (j % 2)
                        co = BLK * (j // 2)
                        nc.tensor.matmul(
                            ek[:, 64 * j:64 * (j + 1)],
                            lhsT=ea_sb[pb:pb + 64, co:co + BLK],
                            rhs=w_e[pb:pb + 64, :], start=True, stop=False)
                        nc.tensor.matmul(
                            ek[:, 64 * j:64 * (j + 1)],
                            lhsT=ind_n[:, 128 * j:128 * (j + 1)],
                            rhs=k_all[:, 64 * b:64 * (b + 1)], start=False, stop=True)

                    q_ap = qv_g[:].rearrange("p (t d) -> p t d", t=rt)[:, j0:j0 + k, 0:64]
                    v_ap = qv_g[:].rearrange("p (t d) -> p t d", t=rt)[:, j0:j0 + k, 64:128]
                    gate_pre = work.tile([128, 64 * k], bf16, tag="gpre")
                    nc.vector.tensor_tensor(out=gate_pre[:], in0=ek[:], in1=q_ap, op=AOT.add)
                    gate = work.tile([128, 64 * k], bf16, tag="gate")
                    nc.scalar.activation(out=gate[:], in_=gate_pre[:], func=AFT.Sigmoid)
                    msg = work.tile([128, 64 * k], bf16, tag="msg")
                    nc.vector.tensor_tensor(out=msg[:], in0=gate[:], in1=v_ap, op=AOT.mult)

                    for j in range(k):
                        g = gt0 + j
                        b = int(tile_block[g])
                        if b not in agg_tiles:
                            agg_t = aggps.tile([128, 64], f32, tag=f"agg{b % (SUPER + 1)}")
                            agg_tiles[b] = agg_t
                        nc.tensor.matmul(
                            agg_tiles[b][:],
                            lhsT=ind_T[:, 128 * j:128 * (j + 1)],
                            rhs=msg[:, 64 * j:64 * (j + 1)],
                            start=(g == first_tile_of_block[b]),
                            stop=(g == last_tile_of_block[b]))
                        if g == last_tile_of_block[b]:
                            sb = b // SUPER
                            if sb not in fin_tiles:
                                fin_t = finp.tile([128, 64 * SUPER], f32, tag="fin")
                                fin_tiles[sb] = fin_t
                            fb = 64 * (b % SUPER)
                            acc_v = fin_tiles[sb][:, fb:fb + 64]
                            nc.vector.tensor_tensor(
                                out=acc_v, in0=agg_tiles[b][:],
                                in1=skip_all[:, 64 * b:64 * (b + 1)], op=AOT.add)
                            nc.scalar.activation(out=acc_v, in_=acc_v, func=AFT.Relu)
                            del agg_tiles[b]
                            done = all((sb * SUPER + i) in (b,)
                                       or ((sb * SUPER + i) not in agg_tiles
                                           and last_tile_of_block.get(
                                               sb * SUPER + i, -1) <= g)
                                       for i in range(SUPER)
                                       if sb * SUPER + i < nblk)
                            if done:
                                kb = min(SUPER, nblk - sb * SUPER)
                                dma(out_d[128 * SUPER * sb:
                                          128 * (SUPER * sb + kb), :].rearrange(
                                              "(j p) c -> p j c", p=128),
                                    fin_tiles[sb][:, :64 * kb].rearrange(
                                        "p (j c) -> p j c", j=kb))
                                del fin_tiles[sb]
                g0 += rt

    nc.compile()
    return nc


# ================================================================== kernel
def kernel(x, edge_index, edge_attr, u,
           W_key, b_key, W_query, b_query, W_value, b_value,
           W_edge, b_edge, W_skip, b_skip):
    import ml_dtypes
    bf = ml_dtypes.bfloat16
    from concourse.bass_utils import run_bass_kernel_spmd

    x = np.asarray(x, dtype=np.float32)
    edge_index = np.asarray(edge_index)
    edge_attr = np.asarray(edge_attr, dtype=np.float32)
    n_nodes = x.shape[0]

    src = np.asarray(edge_index[0], dtype=np.int64)
    dst = np.asarray(edge_index[1], dtype=np.int64)

    sched = _make_schedule(src, dst, n_nodes, NCORES)
    npc, nblk, NT = sched["npc"], sched["nblk"], sched["NT"]
    NPADT = _ceil_div(n_nodes, 128) * 128
    NPADL = nblk * BLK

    x_pad = np.zeros((NPADT, 128), dtype=np.float32)
    x_pad[:n_nodes] = x
    xt_full = np.ascontiguousarray(x_pad.T)        # [128, NPADT]

    W_qv = np.concatenate([np.asarray(W_query, np.float32),
                           np.asarray(W_value, np.float32)], axis=1)
    g_bias = (np.asarray(b_key, np.float32) + np.asarray(b_query, np.float32)
              + np.asarray(b_edge, np.float32)).reshape(1, 64)
    qv_bias = np.concatenate([np.zeros(64, np.float32),
                              np.asarray(b_value, np.float32)]).reshape(1, 128)
    zero_bias = not np.any(qv_bias)

    edge_attr_sorted = edge_attr[sched["order"]]

    packs = [_pack_core(sched, c, edge_attr_sorted) for c in range(NCORES)]

    cfg = dict(NT=NT, nblk=nblk, nsuper=sched["nsuper"], runs=sched["runs"],
               chunks=sched["chunks"], tile_block=sched["tile_block"],
               NPADT=NPADT, zero_bias=zero_bias,
               idx_words=packs[0]["idx16"].shape[1])

    key = (NT, nblk, tuple(sched["runs"]), zero_bias)
    if key not in _graph_cache:
        _graph_cache[key] = _build_graph(cfg)
    nc = _graph_cache[key]

    in_maps = []
    for c in range(NCORES):
        p = packs[c]
        lo = c * npc
        xl_pad = np.zeros((NPADL, 128), dtype=np.float32)
        xl_pad[:npc] = x[lo:lo + npc]
        xl_full = np.ascontiguousarray(xl_pad.T)   # [128, NPADL]
        in_maps.append({
            "xt_full": xt_full.astype(bf),
            "xl_full": xl_full.astype(bf),
            "wqv": W_qv.astype(bf),
            "wk": np.asarray(W_key, np.float32).astype(bf),
            "wskip": np.asarray(W_skip, np.float32).astype(bf),
            "wedge": np.asarray(W_edge, np.float32).astype(bf),
            "gbias": g_bias.astype(bf),
            "sbias": np.asarray(b_skip, np.float32).reshape(1, 64).astype(bf),
            "qvbias": qv_bias.astype(np.float32),
            "eac": p["eac"].astype(bf),
            "idx16": p["idx16"],
            "dofr": p["dofr"],
            "dstrow": p["dstrow"].astype(bf),
        })

    if SIM_MODE:
        import concourse.bass_interp as bass_interp
        sim_cores = list(range(SIM_NCORES))
        sim = bass_interp.MultiCoreSim(nc, len(sim_cores))
        for i in sim_cores:
            for name, arr in in_maps[i].items():
                sim.cores[i].tensor(name)[:] = arr
        sim.simulate()
        results = [{"out": np.asarray(sim.cores[i].tensor("out"))} for i in sim_cores]
        outs = [results[c]["out"][:npc] for c in sim_cores]
        out = np.concatenate(outs, axis=0).astype(np.float32)
        full = np.zeros((n_nodes, 64), np.float32)
        full[:len(sim_cores) * npc] = out[:min(len(out), n_nodes)]
        return (full, edge_attr, u)

    global LAST_RESULT
    res = run_bass_kernel_spmd(nc, in_maps, core_ids=list(range(NCORES)),
                               trace=TRACE)
    LAST_RESULT = res
    outs = [res.results[c]["out"][:npc] for c in range(NCORES)]
    out = np.concatenate(outs, axis=0)[:n_nodes].astype(np.float32)
    return (out, edge_attr, u)


# revision 31
# speedup vs baseline: 1.0299x; 1.0299x over previous
"""Trainium2 Bass kernel for gated GNN message passing (8 NeuronCores, SPMD).

Reference computation:
    k = x @ W_key + b_key;  q = x @ W_query + b_query;  v = x @ W_value + b_value
    e = edge_attr @ W_edge + b_edge
    gate = sigmoid(k[dst] + q[src] + e)
    msg = gate * v[src]
    agg = segment_sum(msg, dst, N)
    out = relu(agg + x @ W_skip + b_skip)
    return (out, edge_attr, u)

Distribution: nodes (and their in-edges, partitioned by destination) are
sharded across 8 cores; weights replicated; source-node features fetched via
an on-device dma_gather from a per-core replica of a (q|v) table built on
device.  No collectives are needed.

Device pipeline per core (edge-major, bf16 compute / f32 accumulate):
  phase 0: qv table [N,128] f32 in DRAM  (q|v rows, built from x via PE)
           k-blocks (+combined gate bias) and skip-blocks (+b_skip) in SBUF
  phase 1: edges grouped by (dst-block of 128 nodes, src window of 25056) and
           padded to 128-edge tiles; per (super-block, window) run one
           dma_gather of qv[src] rows (int16 idx, 4 SWDGE queues);
           per 128-edge tile: e+kd accumulated in PSUM via two matmuls
           (edge_attr^T half-split stationary; node-indicator x k_blk);
           gate = sigmoid(psum + q); msg = gate * v;
           agg += ind_T^T @ msg  (indicator matmul segment sum);
           per super-block: out rows = relu(agg + skip) -> DRAM.
"""

import sys

import numpy as np

try:
    import concourse.bass  # noqa: F401
except ImportError:
    sys.path.insert(0, "/opt/trn_rl_repo")

# ---------------------------------------------------------------- constants
N, E, IN_C, OUT_C, EDGE_D = 100000, 1600000, 128, 64, 64
NCORES = 8
BLK = 128            # dst nodes per block
SUPER = 4            # dst blocks per super-block (gather batching)
NWIN = 4             # source-index windows (int16 reach 32768)
WIN_STRIDE = 25056   # window w covers src in [w*WIN_STRIDE, w*WIN_STRIDE+32768)
WIN_SIZE = 32768
CHUNK_TILES = 8      # tiles per elementwise chunk
NSWQ = 4             # SWDGE queues for dma_gather

_graph_cache = {}
SIM_MODE = False
SIM_NCORES = 1
TRACE = False
LAST_RESULT = None


def _ceil_div(a, b):
    return -(-a // b)


# =================================================================== schedule
def _make_schedule(src, dst, n_nodes, n_cores):
    """Partition edges by destination across cores; build the shared static
    schedule (same instruction stream for every core) plus per-core data."""
    npc = n_nodes // n_cores            # nodes per core
    nblk = _ceil_div(npc, BLK)          # dst blocks per core
    nsuper = _ceil_div(nblk, SUPER)

    order = np.argsort(dst, kind="stable")
    dst_s = dst[order]
    src_s = src[order]

    core_lo = np.searchsorted(dst_s, np.arange(n_cores) * npc)
    core_hi = np.searchsorted(dst_s, (np.arange(n_cores) + 1) * npc)

    # per-core per-block edge lists sorted by src, with window boundary CDFs.
    # Window w accepts src in [w*WIN_STRIDE, w*WIN_STRIDE + WIN_SIZE); edges in
    # the overlap zones are assigned flexibly so that windows 0..NWIN-2 hold
    # exact multiples of 128 edges (minimal padding).
    blk_edges = [[None] * nblk for _ in range(n_cores)]   # src-sorted edge ids
    cnt_cb = np.zeros((n_cores, nblk), dtype=np.int64)
    Lc = np.zeros((n_cores, nblk, NWIN + 1), dtype=np.int64)  # srcs < w*STRIDE
    for c in range(n_cores):
        lo, hi = core_lo[c], core_hi[c]
        d_loc = dst_s[lo:hi] - c * npc
        s_loc = src_s[lo:hi]
        blk_id = d_loc // BLK
        ordk = np.argsort(blk_id * (n_nodes + 1) + s_loc, kind="stable")
        keys_sorted = blk_id[ordk]
        bounds = np.searchsorted(keys_sorted, np.arange(nblk + 1))
        for b in range(nblk):
            sel = lo + ordk[bounds[b]:bounds[b + 1]]
            blk_edges[c][b] = sel
            cnt_cb[c, b] = len(sel)
            srcs = src_s[sel]
            for w in range(NWIN + 1):
                Lc[c, b, w] = np.searchsorted(srcs, w * WIN_STRIDE)

    T_b = _ceil_div(cnt_cb.max(axis=0), BLK)              # tiles per block
    # uniform cumulative tile cut points C[b, w], w = 0..NWIN-1
    t_bw = np.zeros((nblk, NWIN), dtype=np.int64)
    for b in range(nblk):
        C_prev = 0
        for w in range(NWIN - 1):
            C_w = max(int(_ceil_div(Lc[:, b, w + 1].max(), BLK)), C_prev)
            C_w = min(C_w, int(T_b[b]))
            t_bw[b, w] = C_w - C_prev
            C_prev = C_w
        t_bw[b, NWIN - 1] = int(T_b[b]) - C_prev

    # per-core slot filling: window w takes up to 128*t_bw[b,w] eligible edges
    # (src < w*WIN_STRIDE + WIN_SIZE) in src order; shortfall = dummy slots.
    groups = [[[None] * NWIN for _ in range(nblk)] for _ in range(n_cores)]
    for c in range(n_cores):
        for b in range(nblk):
            sel = blk_edges[c][b]
            srcs = src_s[sel]
            ptr = 0
            for w in range(NWIN):
                slots = BLK * int(t_bw[b, w])
                hi_b = w * WIN_STRIDE + WIN_SIZE
                n_elig = int(np.searchsorted(srcs, hi_b)) - ptr
                take = min(slots, n_elig)
                if ptr < len(srcs) and take < min(slots, len(srcs) - ptr):
                    # leftover edges below next window base would be dropped
                    assert srcs[ptr + take] >= (w + 1) * WIN_STRIDE, (
                        "window packing infeasible")
                groups[c][b][w] = sel[ptr:ptr + take]
                ptr += take
            assert ptr == len(sel), (c, b, ptr, len(sel))

    runs = []          # (super, window, n_tiles)
    tile_block = []    # local dst-block per tile
    nt = 0
    for s in range(nsuper):
        blocks = range(s * SUPER, min((s + 1) * SUPER, nblk))
        for w in range(NWIN):
            rt = 0
            for b in blocks:
                tb = int(t_bw[b, w])
                tile_block.extend([b] * tb)
                nt += tb
                rt += tb
            runs.append((s, w, rt))
    tile_block = np.asarray(tile_block, dtype=np.int64)

    # chunks: groups of <=CHUNK_TILES tiles within each run
    chunks = []        # (run_idx, gt0, k)
    g0 = 0
    for ri, (s, w, rt) in enumerate(runs):
        for j0 in range(0, rt, CHUNK_TILES):
            chunks.append((ri, g0 + j0, min(CHUNK_TILES, rt - j0)))
        g0 += rt

    return dict(
        npc=npc, nblk=nblk, nsuper=nsuper, n_nodes=n_nodes,
        runs=runs, tile_block=tile_block, NT=nt, t_bw=t_bw, chunks=chunks,
        groups=groups, order=order, src_s=src_s, dst_s=dst_s,
    )


def _pack_core(sched, c, edge_attr_s):
    """Build one core's input arrays (contiguous pre-tiled layouts)."""
    npc, nblk = sched["npc"], sched["nblk"]
    NT = sched["NT"]
    EPAD = NT * BLK
    t_bw = sched["t_bw"]
    groups = sched["groups"]
    src_s, dst_s = sched["src_s"], sched["dst_s"]
    runs, chunks = sched["runs"], sched["chunks"]

    slot_src = np.zeros(EPAD, dtype=np.int64)
    slot_off = np.full(EPAD, 200.0, dtype=np.float32)
    slot_edge = np.full(EPAD, -1, dtype=np.int64)

    pos = 0
    for s in range(sched["nsuper"]):
        blocks = range(s * SUPER, min((s + 1) * SUPER, nblk))
        for w in range(NWIN):
            for b in blocks:
                sel = groups[c][b][w]
                n = len(sel)
                tb = int(t_bw[b, w])
                slot_src[pos:pos + n] = src_s[sel]
                slot_off[pos:pos + n] = (dst_s[sel] - c * npc - b * BLK).astype(np.float32)
                slot_edge[pos:pos + n] = sel
                slot_src[pos + n:pos + tb * BLK] = min(w * WIN_STRIDE,
                                                       sched["n_nodes"] - 1)
                pos += tb * BLK
    assert pos == EPAD

    # ---- gather indices: per-run contiguous [128, n/16] C-order blocks ----
    idx_words = sum(128 * (rt * BLK // 16) for (_, _, rt) in runs)
    idx16 = np.zeros(idx_words, dtype=np.int16)
    off = 0
    pos = 0
    for (s, w, rt) in runs:
        n = rt * BLK
        if n:
            local = (slot_src[pos:pos + n] - w * WIN_STRIDE).astype(np.int64)
            assert local.min() >= 0 and local.max() < WIN_SIZE
            wrapped = local.reshape(n // 16, 16).T.astype(np.int16)   # [16, n/16]
            block = np.tile(wrapped, (8, 1))                          # [128, n/16]
            idx16[off:off + 128 * (n // 16)] = block.ravel()
            off += 128 * (n // 16)
        pos += n
    assert off == idx_words and pos == EPAD

    # ---- dst offsets: per-run contiguous [128, rt] C-order ---------------
    dofr = np.zeros(128 * NT, dtype=np.float32)
    offv = slot_off.reshape(NT, BLK).T        # [128, NT]
    off = 0
    g0 = 0
    for (s, w, rt) in runs:
        if rt:
            dofr[off:off + 128 * rt] = offv[:, g0:g0 + rt].ravel()
            off += 128 * rt
        g0 += rt
    dstrow = slot_off.reshape(1, EPAD)        # edge-major (broadcast source)

    # ---- edge_attr: per-chunk [128, 256] half-split contiguous ----------
    nchunk = len(chunks)
    eac = np.zeros((nchunk * 128, (CHUNK_TILES // 2) * BLK), dtype=np.float32)
    valid = slot_edge >= 0
    ea_rows = np.zeros((EPAD, EDGE_D), dtype=np.float32)
    ea_rows[valid] = edge_attr_s[slot_edge[valid]]
    eaT = ea_rows.reshape(NT, BLK, EDGE_D).transpose(0, 2, 1)  # [NT, 64ch, 128p]
    for ci, (ri, gt0, k) in enumerate(chunks):
        blkv = eac[128 * ci:128 * ci + 128].reshape(128, CHUNK_TILES // 2, BLK)
        for j in range(k):
            blkv[64 * (j % 2):64 * (j % 2) + 64, j // 2, :] = eaT[gt0 + j]

    return dict(
        idx16=idx16.reshape(1, -1),
        dofr=dofr.reshape(1, -1),
        dstrow=dstrow,
        eac=eac,
        EPAD=EPAD,
    )


# ================================================================ bass graph
def _build_graph(cfg):
    import concourse.bass as bass  # noqa: F401
    from concourse import bacc
    import concourse.mybir as mybir
    from concourse.tile import TileContext

    f32 = mybir.dt.float32
    bf16 = mybir.dt.bfloat16
    i32 = mybir.dt.int32
    i16 = mybir.dt.int16
    AOT = mybir.AluOpType
    AFT = mybir.ActivationFunctionType

    NT = cfg["NT"]
    nblk = cfg["nblk"]
    runs = cfg["runs"]
    chunks = cfg["chunks"]
    tile_block = cfg["tile_block"]
    NPADT = cfg["NPADT"]
    NPADL = nblk * BLK
    idx_words = cfg["idx_words"]
    nchunk = len(chunks)
    zero_bias = cfg["zero_bias"]

    nc = bacc.Bacc(num_swdge_queues=NSWQ)

    # ---------------- dram parameters ----------------
    xt_t = nc.declare_dram_parameter("xt_full", [128, NPADT], bf16, isOutput=False)
    xl_t = nc.declare_dram_parameter("xl_full", [128, NPADL], bf16, isOutput=False)
    wqv = nc.declare_dram_parameter("wqv", [128, 128], bf16, isOutput=False)
    wk = nc.declare_dram_parameter("wk", [128, 64], bf16, isOutput=False)
    wskip = nc.declare_dram_parameter("wskip", [128, 64], bf16, isOutput=False)
    wedge = nc.declare_dram_parameter("wedge", [64, 64], bf16, isOutput=False)
    gbias = nc.declare_dram_parameter("gbias", [1, 64], bf16, isOutput=False)
    sbias = nc.declare_dram_parameter("sbias", [1, 64], bf16, isOutput=False)
    qvbias = nc.declare_dram_parameter("qvbias", [1, 128], f32, isOutput=False)
    eac_d = nc.declare_dram_parameter("eac", [nchunk * 128, (CHUNK_TILES // 2) * BLK], bf16, isOutput=False)
    idx_d = nc.declare_dram_parameter("idx16", [1, idx_words], i16, isOutput=False)
    dofr_d = nc.declare_dram_parameter("dofr", [1, 128 * NT], f32, isOutput=False)
    drow_d = nc.declare_dram_parameter("dstrow", [1, NT * BLK], bf16, isOutput=False)
    out_d = nc.declare_dram_parameter("out", [NPADL, 64], f32, isOutput=True)

    qvtab = nc.dram_tensor("qvtab", [NPADT, 128], f32)

    # alternate HWDGE issue between SP and ACT sequencers
    dma_ctr = [0]

    def dma(out, in_):
        dma_ctr[0] += 1
        eng = nc.sync if dma_ctr[0] % 2 else nc.scalar
        return eng.dma_start(out=out, in_=in_)

    with TileContext(nc) as tc, tc.tile_pool(name="consts", bufs=1) as cpool:
        w_qv = cpool.tile([128, 128], bf16); nc.sync.dma_start(out=w_qv[:], in_=wqv[:])
        w_k = cpool.tile([128, 64], bf16); nc.sync.dma_start(out=w_k[:], in_=wk[:])
        w_s = cpool.tile([128, 64], bf16); nc.sync.dma_start(out=w_s[:], in_=wskip[:])
        w_e = cpool.tile([128, 64], bf16)
        nc.sync.dma_start(out=w_e[0:64, :], in_=wedge[:])
        nc.sync.dma_start(out=w_e[64:128, :], in_=wedge[:])
        gb = cpool.tile([1, 64], bf16); nc.sync.dma_start(out=gb[:], in_=gbias[:])
        sb_b = cpool.tile([1, 64], bf16); nc.sync.dma_start(out=sb_b[:], in_=sbias[:])
        ones1 = cpool.tile([1, 128], bf16); nc.vector.memset(ones1[:], 1.0)
        qvb = cpool.tile([128, 128], f32)
        nc.sync.dma_start(out=qvb[:], in_=qvbias[:].to_broadcast([128, 128]))
        iota_i = cpool.tile([128, 128], i32)
        nc.gpsimd.iota(iota_i[:], pattern=[[1, 128]], base=0, channel_multiplier=0)
        iota_row = cpool.tile([128, 128], bf16)
        nc.vector.tensor_copy(out=iota_row[:], in_=iota_i[:])
        ioc_i = cpool.tile([128, 1], i32)
        nc.gpsimd.iota(ioc_i[:], pattern=[[0, 1]], base=0, channel_multiplier=1)
        iota_col = cpool.tile([128, 1], bf16)
        nc.vector.tensor_copy(out=iota_col[:], in_=ioc_i[:])

        k_all = cpool.tile([128, 64 * nblk], bf16)
        skip_all = cpool.tile([128, 64 * nblk], f32)

        # ---------------- phase 0a: qv table (groups of 4 blocks) ----------
        tab_writes = []   # (row_hi, dma_inst)
        ntb = NPADT // 128
        with tc.tile_pool(name="p0", bufs=3) as p0, \
             tc.tile_pool(name="p0ps", bufs=3, space="PSUM") as p0ps:
            for tq in range(_ceil_div(ntb, 4)):
                t0 = tq * 4
                kt = min(4, ntb - t0)
                xb = p0.tile([128, 128 * kt], bf16, tag="xb")
                dma(xb[:], xt_t[:, 128 * t0:128 * (t0 + kt)])
                ps = p0ps.tile([128, 128 * kt], f32, tag="ps")
                for j in range(kt):
                    nc.tensor.matmul(ps[:, 128 * j:128 * (j + 1)],
                                     lhsT=xb[:, 128 * j:128 * (j + 1)],
                                     rhs=w_qv[:], start=True, stop=True)
                sbuf = p0.tile([128, 128 * kt], f32, tag="qvsb")
                if zero_bias:
                    if tq % 2 == 0:
                        nc.scalar.activation(out=sbuf[:], in_=ps[:], func=AFT.Copy)
                    else:
                        nc.vector.tensor_copy(out=sbuf[:], in_=ps[:])
                else:
                    for j in range(kt):
                        nc.vector.tensor_tensor(out=sbuf[:, 128 * j:128 * (j + 1)],
                                                in0=ps[:, 128 * j:128 * (j + 1)],
                                                in1=qvb[:], op=AOT.add)
                wi = dma(qvtab[128 * t0:128 * (t0 + kt), :].rearrange(
                    "(t n) c -> n t c", t=kt),
                    sbuf[:].rearrange("n (t c) -> n t c", t=kt))
                tab_writes.append((128 * (t0 + kt), wi))

        # per-window funnel markers: gathers of window w wait on all table
        # writes covering [w*WIN_STRIDE, w*WIN_STRIDE+WIN_SIZE)
        from concourse.tile import add_dep_helper
        win_markers = []
        mk_tile = cpool.tile([1, 8], f32)
        for w in range(NWIN):
            hi_w = min(w * WIN_STRIDE + WIN_SIZE, NPADT)
            mk = nc.vector.memset(mk_tile[0:1, w:w + 1], 0.0)
            for row_hi, wi in tab_writes:
                if True if row_hi <= hi_w else False:
                    add_dep_helper(mk.ins, wi.ins, sync=True,
                                   reason=f"qvtab window {w} ready")
            win_markers.append(mk)

        # ---------------- phase 0b: k & skip blocks (groups of 4) ----------
        with tc.tile_pool(name="p0b", bufs=3) as p0b, \
             tc.tile_pool(name="p0bps", bufs=2, space="PSUM") as p0bps:
            for bq in range(_ceil_div(nblk, 4)):
                b0 = bq * 4
                kb = min(4, nblk - b0)
                xb = p0b.tile([128, 128 * kb], bf16, tag="xlb")
                dma(xb[:], xl_t[:, 128 * b0:128 * (b0 + kb)])
                kps = p0bps.tile([128, 64 * kb], f32, tag="kps")
                sps = p0bps.tile([128, 64 * kb], f32, tag="sps")
                for j in range(kb):
                    nc.tensor.matmul(kps[:, 64 * j:64 * (j + 1)],
                                     lhsT=xb[:, 128 * j:128 * (j + 1)],
                                     rhs=w_k[:], start=True, stop=False)
                    nc.tensor.matmul(kps[:, 64 * j:64 * (j + 1)],
                                     lhsT=ones1[:], rhs=gb[:], start=False, stop=True)
                    nc.tensor.matmul(sps[:, 64 * j:64 * (j + 1)],
                                     lhsT=xb[:, 128 * j:128 * (j + 1)],
                                     rhs=w_s[:], start=True, stop=False)
                    nc.tensor.matmul(sps[:, 64 * j:64 * (j + 1)],
                                     lhsT=ones1[:], rhs=sb_b[:], start=False, stop=True)
                nc.vector.tensor_copy(out=k_all[:, 64 * b0:64 * (b0 + kb)], in_=kps[:])
                nc.scalar.activation(out=skip_all[:, 64 * b0:64 * (b0 + kb)],
                                     in_=sps[:], func=AFT.Copy)

        # ---------------- phase 1: edge pipeline ----------------
        first_tile_of_block = {}
        last_tile_of_block = {}
        for g, b in enumerate(tile_block):
            first_tile_of_block.setdefault(int(b), g)
            last_tile_of_block[int(b)] = g

        run_chunks = {}
        for ci, (ri, gt0, k) in enumerate(chunks):
            run_chunks.setdefault(ri, []).append((ci, gt0, k))

        with tc.tile_pool(name="gq", bufs=4) as gq, \
             tc.tile_pool(name="work", bufs=4) as work, \
             tc.tile_pool(name="ind", bufs=4) as indp, \
             tc.tile_pool(name="finp", bufs=2) as finp, \
             tc.tile_pool(name="eps", bufs=3, space="PSUM") as eps, \
             tc.tile_pool(name="aggps", bufs=1, space="PSUM") as aggps:

            agg_tiles = {}
            fin_tiles = {}
            idx_off = 0
            dof_off = 0
            g0 = 0
            qnum = 0
            for ri, (s, w, rt) in enumerate(runs):
                if rt == 0:
                    continue
                n_idx = rt * BLK
                ixt = work.tile([128, n_idx // 16], i16, tag="ixt")
                dma(ixt[:], idx_d[0, idx_off:idx_off + 128 * (n_idx // 16)].rearrange(
                    "(p c) -> p c", p=128))
                idx_off += 128 * (n_idx // 16)
                dofr_sb = work.tile([128, rt], f32, tag="dofr")
                dma(dofr_sb[:], dofr_d[0, dof_off:dof_off + 128 * rt].rearrange(
                    "(p c) -> p c", p=128))
                dof_off += 128 * rt

                qv_g = gq.tile([128, rt * 128], f32, tag="qvg")
                win_hi = min(w * WIN_STRIDE + WIN_SIZE, NPADT)
                # split the run's gather across the SWDGE queues so their
                # descriptor generation overlaps on the Q7 cores
                nsub = min(NSWQ, rt)
                sub = _ceil_div(rt, nsub)
                t0s = list(range(0, rt, sub))
                for si, st in enumerate(t0s):
                    en = min(st + sub, rt)
                    ni = (en - st) * BLK
                    gi = nc.gpsimd.dma_gather(
                        out_ap=qv_g[:, 128 * st:128 * en].rearrange(
                            "p (t d) -> p t d", t=en - st),
                        in_ap=qvtab[w * WIN_STRIDE:win_hi, :],
                        idxs_ap=ixt[:, 8 * st:8 * en],
                        num_idxs=ni, num_idxs_reg=ni, elem_size=128,
                        single_packet=False, queue_num=(qnum + si) % NSWQ)
                    from concourse.tile import add_dep_helper as _adh
                    _adh(gi.ins if hasattr(gi, 'ins') else gi, win_markers[w].ins, sync=True,
                         reason="gather after qvtab window ready")
                qnum += len(t0s)

                dbc = gq.tile([128, rt * 128], bf16, tag="dbc")
                dma(dbc[:], drow_d[:, BLK * g0:BLK * (g0 + rt)].to_broadcast(
                    [128, rt * 128]))

                ea_pair = {}
                rcs = run_chunks[ri]
                for pi in range(0, len(rcs), 2):
                    pcs = rcs[pi:pi + 2]
                    u = len(pcs)
                    et = work.tile([128, u * (CHUNK_TILES // 2) * BLK], bf16,
                                   tag="easb")
                    ci0 = pcs[0][0]
                    dma(et[:].rearrange("p (u c) -> p u c", u=u),
                        eac_d[128 * ci0:128 * (ci0 + u), :].rearrange(
                            "(u p) c -> p u c", u=u))
                    for ui, (ci, _, _) in enumerate(pcs):
                        ea_pair[ci] = (et, ui)
                for (ci, gt0, k) in run_chunks[ri]:
                    j0 = gt0 - g0
                    ind_n = indp.tile([128, 128 * k], bf16, tag="indn")
                    nc.vector.tensor_tensor(
                        out=ind_n[:], in0=dbc[:, 128 * j0:128 * (j0 + k)],
                        in1=iota_col[:].to_broadcast([128, 128 * k]),
                        op=AOT.is_equal)
                    ind_T = indp.tile([128, 128 * k], bf16, tag="indt")
                    for j in range(k):
                        nc.vector.tensor_tensor(
                            out=ind_T[:, 128 * j:128 * (j + 1)], in0=iota_row[:],
                            in1=dofr_sb[:, j0 + j:j0 + j + 1].to_broadcast([128, 128]),
                            op=AOT.is_equal)

                    et, ui = ea_pair[ci]
                    ea_sb = et[:, ui * (CHUNK_TILES // 2) * BLK:
                               (ui + 1) * (CHUNK_TILES // 2) * BLK]

                    ek = eps.tile([128, 64 * k], f32, tag="ek")
                    for j in range(k):
                        g = gt0 + j
                        b = int(tile_block[g])
                        pb = 64 * (j % 2)
                        co = BLK * (j // 2)
                        nc.tensor.matmul(
                            ek[:, 64 * j:64 * (j + 1)],
                            lhsT=ea_sb[pb:pb + 64, co:co + BLK],
                            rhs=w_e[pb:pb + 64, :], start=True, stop=False)
                        nc.tensor.matmul(
                            ek[:, 64 * j:64 * (j + 1)],
                            lhsT=ind_n[:, 128 * j:128 * (j + 1)],
                            rhs=k_all[:, 64 * b:64 * (b + 1)], start=False, stop=True)

                    q_ap = qv_g[:].rearrange("p (t d) -> p t d", t=rt)[:, j0:j0 + k, 0:64]
                    v_ap = qv_g[:].rearrange("p (t d) -> p t d", t=rt)[:, j0:j0 + k, 64:128]
                    gate_pre = work.tile([128, 64 * k], bf16, tag="gpre")
                    nc.vector.tensor_tensor(out=gate_pre[:], in0=ek[:], in1=q_ap, op=AOT.add)
                    gate = work.tile([128, 64 * k], bf16, tag="gate")
                    nc.scalar.activation(out=gate[:], in_=gate_pre[:], func=AFT.Sigmoid)
                    msg = work.tile([128, 64 * k], bf16, tag="msg")
                    nc.vector.tensor_tensor(out=msg[:], in0=gate[:], in1=v_ap, op=AOT.mult)

                    for j in range(k):
                        g = gt0 + j
                        b = int(tile_block[g])
                        if b not in agg_tiles:
                            agg_t = aggps.tile([128, 64], f32, tag=f"agg{b % (SUPER + 1)}")
                            agg_tiles[b] = agg_t
                        nc.tensor.matmul(
                            agg_tiles[b][:],
                            lhsT=ind_T[:, 128 * j:128 * (j + 1)],
                            rhs=msg[:, 64 * j:64 * (j + 1)],
                            start=(g == first_tile_of_block[b]),
                            stop=(g == last_tile_of_block[b]))
                        if g == last_tile_of_block[b]:
                            sb = b // SUPER
                            if sb not in fin_tiles:
                                fin_t = finp.tile([128, 64 * SUPER], f32, tag="fin")
                                fin_tiles[sb] = fin_t
                            fb = 64 * (b % SUPER)
                            acc_v = fin_tiles[sb][:, fb:fb + 64]
                            nc.vector.tensor_tensor(
                                out=acc_v, in0=agg_tiles[b][:],
                                in1=skip_all[:, 64 * b:64 * (b + 1)], op=AOT.add)
                            nc.scalar.activation(out=acc_v, in_=acc_v, func=AFT.Relu)
                            del agg_tiles[b]
                            done = all((sb * SUPER + i) in (b,)
                                       or ((sb * SUPER + i) not in agg_tiles
                                           and last_tile_of_block.get(
                                               sb * SUPER + i, -1) <= g)
                                       for i in range(SUPER)
                                       if sb * SUPER + i < nblk)
                            if done:
                                kb = min(SUPER, nblk - sb * SUPER)
                                dma(out_d[128 * SUPER * sb:
                                          128 * (SUPER * sb + kb), :].rearrange(
                                              "(j p) c -> p j c", p=128),
                                    fin_tiles[sb][:, :64 * kb].rearrange(
                                        "p (j c) -> p j c", j=kb))
                                del fin_tiles[sb]
                g0 += rt

    nc.compile()
    return nc


# ================================================================== kernel
def kernel(x, edge_index, edge_attr, u,
           W_key, b_key, W_query, b_query, W_value, b_value,
           W_edge, b_edge, W_skip, b_skip):
    import ml_dtypes
    bf = ml_dtypes.bfloat16
    from concourse.bass_utils import run_bass_kernel_spmd

    x = np.asarray(x, dtype=np.float32)
    edge_index = np.asarray(edge_index)
    edge_attr = np.asarray(edge_attr, dtype=np.float32)
    n_nodes = x.shape[0]

    src = np.asarray(edge_index[0], dtype=np.int64)
    dst = np.asarray(edge_index[1], dtype=np.int64)

    sched = _make_schedule(src, dst, n_nodes, NCORES)
    npc, nblk, NT = sched["npc"], sched["nblk"], sched["NT"]
    NPADT = _ceil_div(n_nodes, 128) * 128
    NPADL = nblk * BLK

    x_pad = np.zeros((NPADT, 128), dtype=np.float32)
    x_pad[:n_nodes] = x
    xt_full = np.ascontiguousarray(x_pad.T)        # [128, NPADT]

    W_qv = np.concatenate([np.asarray(W_query, np.float32),
                           np.asarray(W_value, np.float32)], axis=1)
    g_bias = (np.asarray(b_key, np.float32) + np.asarray(b_query, np.float32)
              + np.asarray(b_edge, np.float32)).reshape(1, 64)
    qv_bias = np.concatenate([np.zeros(64, np.float32),
                              np.asarray(b_value, np.float32)]).reshape(1, 128)
    zero_bias = not np.any(qv_bias)

    edge_attr_sorted = edge_attr[sched["order"]]

    packs = [_pack_core(sched, c, edge_attr_sorted) for c in range(NCORES)]

    cfg = dict(NT=NT, nblk=nblk, nsuper=sched["nsuper"], runs=sched["runs"],
               chunks=sched["chunks"], tile_block=sched["tile_block"],
               NPADT=NPADT, zero_bias=zero_bias,
               idx_words=packs[0]["idx16"].shape[1])

    key = (NT, nblk, tuple(sched["runs"]), zero_bias)
    if key not in _graph_cache:
        _graph_cache[key] = _build_graph(cfg)
    nc = _graph_cache[key]

    in_maps = []
    for c in range(NCORES):
        p = packs[c]
        lo = c * npc
        xl_pad = np.zeros((NPADL, 128), dtype=np.float32)
        xl_pad[:npc] = x[lo:lo + npc]
        xl_full = np.ascontiguousarray(xl_pad.T)   # [128, NPADL]
        in_maps.append({
            "xt_full": xt_full.astype(bf),
            "xl_full": xl_full.astype(bf),
            "wqv": W_qv.astype(bf),
            "wk": np.asarray(W_key, np.float32).astype(bf),
            "wskip": np.asarray(W_skip, np.float32).astype(bf),
            "wedge": np.asarray(W_edge, np.float32).astype(bf),
            "gbias": g_bias.astype(bf),
            "sbias": np.asarray(b_skip, np.float32).reshape(1, 64).astype(bf),
            "qvbias": qv_bias.astype(np.float32),
            "eac": p["eac"].astype(bf),
            "idx16": p["idx16"],
            "dofr": p["dofr"],
            "dstrow": p["dstrow"].astype(bf),
        })

    if SIM_MODE:
        import concourse.bass_interp as bass_interp
        sim_cores = list(range(SIM_NCORES))
        sim = bass_interp.MultiCoreSim(nc, len(sim_cores))
        for i in sim_cores:
            for name, arr in in_maps[i].items():
                sim.cores[i].tensor(name)[:] = arr
        sim.simulate()
        results = [{"out": np.asarray(sim.cores[i].tensor("out"))} for i in sim_cores]
        outs = [results[c]["out"][:npc] for c in sim_cores]
        out = np.concatenate(outs, axis=0).astype(np.float32)
        full = np.zeros((n_nodes, 64), np.float32)
        full[:len(sim_cores) * npc] = out[:min(len(out), n_nodes)]
        return (full, edge_attr, u)

    global LAST_RESULT
    res = run_bass_kernel_spmd(nc, in_maps, core_ids=list(range(NCORES)),
                               trace=TRACE)
    LAST_RESULT = res
    outs = [res.results[c]["out"][:npc] for c in range(NCORES)]
    out = np.concatenate(outs, axis=0)[:n_nodes].astype(np.float32)
    return (out, edge_attr, u)
